# revision 7
# baseline (speedup 1.0000x reference)
"""AdaptiveGraphPooling on 8 TRN2 NeuronCores.

kernel(**inputs) -> (x_kept, new_ei, ew, mask, batch_kept, indices)

Device NEFF-1: per-node-shard attention scores (PE transpose + matmuls + relu).
Host:          monotone-key argsort of the 100k scores (rank tables).
Device NEFF-2: all heavy data movement — 6.4M-element edge remap gathers of the
               rank table, mask/ew computation, 50k x-row gathers (x_kept),
               batch gathers — sharded across the 8 cores.
"""
import numpy as np

N = 100000
D = 256
H = 64
E = 3200000
K = 50000
NCORES = 8
NPAD = 100352            # 8 * 12544
SH = NPAD // NCORES      # 12544 nodes/core
NT = SH // 128           # 98 node tiles/core
EPAD = 3211264           # 8 * 401408 padded edges
ESH = EPAD // NCORES     # 401408 edges/core
EF = ESH // 128          # 3136
ECH = 25088              # edges per gather chunk (196 cols)
NCHUNK = ESH // ECH      # 16
NSPLIT = 14              # indirect-gather instruction split per chunk
SPLITC = ECH // NSPLIT   # 1792 descs per instruction
KS = 6272                # kept rows handled per core (first 6250 used)

_BUILT = {}


def _install_profile_hook():
    import sys, types
    import antenv
    try:
        from antenv.axon_hooks import get_axon_ntff_profile_hook
        if get_axon_ntff_profile_hook() is not None:
            return
    except ImportError:
        mod = types.ModuleType("antenv.axon_hooks")
        mod._hook = None
        def _set(hook):
            mod._hook = hook
        def _get():
            return mod._hook
        mod.set_axon_ntff_profile_hook = _set
        mod.get_axon_ntff_profile_hook = _get
        sys.modules["antenv.axon_hooks"] = mod
        antenv.axon_hooks = mod
    try:
        from antenv.axon_hooks import set_axon_ntff_profile_hook
        from trn_agent_boot.trn_boot import _ntff_profile_via_ctypes
        set_axon_ntff_profile_hook(_ntff_profile_via_ctypes('/opt/axon/libaxon_pjrt.so'))
    except Exception:
        pass


def build_scores():
    """NEFF-1: scores[i] = W2 @ relu(W1 @ x_i + b1) + b2 for the core's shard."""
    import concourse.bass as bass
    import concourse.bacc as bacc
    import concourse.mybir as mybir
    f32 = mybir.dt.float32

    nc = bacc.Bacc("TRN2", debug=False)
    x_sh = nc.declare_dram_parameter("x_sh", [SH, D], f32, isOutput=False)
    w1t = nc.declare_dram_parameter("w1t", [D, H], f32, isOutput=False)
    b1_in = nc.declare_dram_parameter("b1_in", [H, 1], f32, isOutput=False)
    w2t = nc.declare_dram_parameter("w2t", [H, 1], f32, isOutput=False)
    b2_in = nc.declare_dram_parameter("b2_in", [1, 1], f32, isOutput=False)
    ident_in = nc.declare_dram_parameter("ident", [128, 128], f32, isOutput=False)
    out_s = nc.declare_dram_parameter("out_s", [128, NT], f32, isOutput=True)

    with (
        nc.Block() as block,
        nc.sbuf_tensor("xa", [128, 256], f32) as xa,
        nc.sbuf_tensor("xb", [128, 256], f32) as xb,
        nc.sbuf_tensor("w1t_sb", [128, 2 * H], f32) as w1t_sb,
        nc.sbuf_tensor("b1_sb", [H, 1], f32) as b1_sb,
        nc.sbuf_tensor("w2t_sb", [H, 1], f32) as w2t_sb,
        nc.sbuf_tensor("b2_sb", [1, 1], f32) as b2_sb,
        nc.sbuf_tensor("id_sb", [128, 128], f32) as id_sb,
        nc.sbuf_tensor("xT_sb", [128, 256], f32) as xT_sb,
        nc.sbuf_tensor("hT_sb", [H, 128], f32) as hT_sb,
        nc.sbuf_tensor("sw", [128, NT], f32) as sw,
        nc.psum_tensor("ps_t", [128, 256], f32) as ps_t,
        nc.psum_tensor("ps_h", [H, 128], f32) as ps_h,
        nc.psum_tensor("ps_s", [128, 1], f32) as ps_s,
        nc.semaphore("ld") as ld,       # x tile loads (sync engine), +16 each
        nc.semaphore("pe") as pe,       # tensor engine progress, +1 steps
        nc.semaphore("dv") as dv,       # vector progress
        nc.semaphore("sc") as sc,       # scalar progress
        nc.semaphore("done") as done,
    ):
        @block.sync
        def _(s):
            s.dma_start(out=w1t_sb[:, 0:H], in_=w1t[0:128, :]).then_inc(ld, 16)
            s.dma_start(out=w1t_sb[:, H:2 * H], in_=w1t[128:256, :]).then_inc(ld, 16)
            s.dma_start(out=b1_sb[:], in_=b1_in[:]).then_inc(ld, 16)
            s.dma_start(out=w2t_sb[:], in_=w2t[:]).then_inc(ld, 16)
            s.dma_start(out=b2_sb[:], in_=b2_in[:]).then_inc(ld, 16)
            s.dma_start(out=id_sb[:], in_=ident_in[:]).then_inc(ld, 16)
            for t in range(NT):
                buf = xa if t % 2 == 0 else xb
                if t >= 2:
                    # wait until PE consumed tile t-2 (transpose done => step 1 of t-2)
                    s.wait_ge(pe, (t - 2) * 4 + 2)
                s.dma_start(out=buf[:], in_=x_sh[t * 128:(t + 1) * 128, :]).then_inc(ld, 16)
            s.wait_ge(dv, 3 * NT)
            s.dma_start(out=out_s[:], in_=sw[:]).then_inc(done, 16)

        @block.tensor
        def _(te):
            for t in range(NT):
                buf = xa if t % 2 == 0 else xb
                te.wait_ge(ld, 96 + (t + 1) * 16)
                if t > 0:
                    te.wait_ge(dv, 3 * (t - 1) + 2)   # ps_t free (xT copies of t-1 done)
                te.transpose(out=ps_t[:, 0:128], in_=buf[:, 0:128], identity=id_sb[:]).then_inc(pe, 1)
                te.transpose(out=ps_t[:, 128:256], in_=buf[:, 128:256], identity=id_sb[:]).then_inc(pe, 1)
                te.wait_ge(dv, t * 3 + 2)             # xT of t ready
                te.matmul(out=ps_h[:], lhsT=w1t_sb[:, 0:H], rhs=xT_sb[:, 0:128], start=True, stop=False)
                te.matmul(out=ps_h[:], lhsT=w1t_sb[:, H:2 * H], rhs=xT_sb[:, 128:256],
                          start=False, stop=True).then_inc(pe, 1)
                te.wait_ge(sc, t + 1)                 # relu of t done
                if t > 0:
                    te.wait_ge(dv, 3 * (t - 1) + 3)   # ps_s free
                te.matmul(out=ps_s[:], lhsT=hT_sb[:], rhs=w2t_sb[:], start=True, stop=True).then_inc(pe, 1)

        @block.vector
        def _(v):
            for t in range(NT):
                v.wait_ge(pe, t * 4 + 2)
                v.tensor_copy(out=xT_sb[:, 0:128], in_=ps_t[:, 0:128]).then_inc(dv, 1)
                v.tensor_copy(out=xT_sb[:, 128:256], in_=ps_t[:, 128:256]).then_inc(dv, 1)
                v.wait_ge(pe, t * 4 + 4)
                v.tensor_copy(out=sw[:, t:t + 1], in_=ps_s[:]).then_inc(dv, 1)

        @block.scalar
        def _(sce):
            import concourse.mybir as mybir
            for t in range(NT):
                sce.wait_ge(pe, t * 4 + 3)
                sce.activation(out=hT_sb[:], in_=ps_h[:],
                               func=mybir.ActivationFunctionType.Relu,
                               bias=b1_sb[:], scale=1.0).then_inc(sc, 1)

    nc.compile()
    return nc


def build_main():
    """NEFF-2: edge remap + mask/ew + x_kept/batch gathers."""
    import concourse.bass as bass
    import concourse.bacc as bacc
    import concourse.mybir as mybir
    from concourse.bass import IndirectOffsetOnAxis
    f32 = mybir.dt.float32
    i32 = mybir.dt.int32
    u8 = mybir.dt.uint8
    Alu = mybir.AluOpType

    nc = bacc.Bacc("TRN2", debug=False)
    x_full = nc.declare_dram_parameter("x_full", [NPAD, D], f32, isOutput=False)
    rtab = nc.declare_dram_parameter("rtab", [NPAD, 1], f32, isOutput=False)  # rank as f32
    batch_all = nc.declare_dram_parameter("batch_all", [NPAD, 1], f32, isOutput=False)
    eu_in = nc.declare_dram_parameter("eu_in", [128, EF], i32, isOutput=False)   # wrapped
    ev_in = nc.declare_dram_parameter("ev_in", [128, EF], i32, isOutput=False)   # wrapped
    ew_in = nc.declare_dram_parameter("ew_in", [ESH], f32, isOutput=False)       # flat
    myidx_in = nc.declare_dram_parameter("myidx_in", [128, KS // 128], i32, isOutput=False)  # wrapped node ids

    out_newu = nc.declare_dram_parameter("out_newu", [ESH], i32, isOutput=True)
    out_newv = nc.declare_dram_parameter("out_newv", [ESH], i32, isOutput=True)
    out_mask = nc.declare_dram_parameter("out_mask", [ESH], u8, isOutput=True)
    out_ew = nc.declare_dram_parameter("out_ew", [ESH], f32, isOutput=True)
    out_xk = nc.declare_dram_parameter("out_xk", [KS, D], f32, isOutput=True)
    out_bk = nc.declare_dram_parameter("out_bk", [1, KS], f32, isOutput=True)

    KC = KS // 128   # 49
    stg_d = nc.dram_tensor("stg_d", [ECH], f32)

    with (
        nc.Block() as block,
        nc.sbuf_tensor("stage", [NSPLIT, SPLITC], f32) as stage,
        nc.sbuf_tensor("eu_sb", [128, EF], i32) as eu_sb,
        nc.sbuf_tensor("ev_sb", [128, EF], i32) as ev_sb,
        nc.sbuf_tensor("ru", [128, 196], f32) as ru,
        nc.sbuf_tensor("rv", [128, 196], f32) as rv,
        nc.sbuf_tensor("mu", [128, 196], f32) as mu,
        nc.sbuf_tensor("mv", [128, 196], f32) as mv,
        nc.sbuf_tensor("t0", [128, 196], f32) as t0,
        nc.sbuf_tensor("nu_i", [128, 196], i32) as nu_i,
        nc.sbuf_tensor("nv_i", [128, 196], i32) as nv_i,
        nc.sbuf_tensor("mk8", [128, 196], u8) as mk8,
        nc.sbuf_tensor("ewc", [128, 196], f32) as ewc,
        nc.sbuf_tensor("ewo", [128, 196], f32) as ewo,
        nc.sbuf_tensor("myidx_sb", [128, KC], i32) as myidx_sb,
        nc.sbuf_tensor("xg", [128, KC, D], f32) as xg,
        nc.sbuf_tensor("bk_sb", [1, KS], f32) as bk_sb,
        nc.semaphore("dsem") as dsem,
        nc.semaphore("gsem") as gsem,
        nc.semaphore("vsem") as vsem,
        nc.semaphore("qsem") as qsem,
    ):
        dc = [0]
        gc = [0]

        @block.gpsimd
        def _(g):
            def dma(out, in_):
                g.dma_start(out=out, in_=in_).then_inc(dsem, 16)
                dc[0] += 16

            dma(eu_sb[:], eu_in[:])
            dma(ev_sb[:], ev_in[:])
            dma(myidx_sb[:], myidx_in[:])
            g.wait_ge(dsem, dc[0])

            # ---- x_kept: 49 row-gather instructions ----
            for j in range(KC):
                g.indirect_dma_start(
                    out=xg[:, j, :], out_offset=None, in_=x_full[:],
                    in_offset=IndirectOffsetOnAxis(ap=myidx_sb[:, j:j + 1], axis=0),
                ).then_inc(gsem, 16)
                gc[0] += 16
            # batch_kept: one [1, KS] gather
            g.indirect_dma_start(
                out=bk_sb[:].rearrange("o f -> o f ()"), out_offset=None,
                in_=batch_all[:],
                in_offset=IndirectOffsetOnAxis(ap=myidx_sb[:, :], axis=0),
            ).then_inc(gsem, 16)
            gc[0] += 16
            g.wait_ge(gsem, gc[0])
            dma(out_bk[:], bk_sb[:])
            dma(out_xk[:].rearrange("(f p) d -> p f d", p=128), xg[:])
            g.wait_ge(dsem, dc[0])

            # ---- edges ----
            for k in range(NCHUNK):
                for which, esb in ((0, eu_sb), (1, ev_sb)):
                    for j in range(NSPLIT):
                        # stream slice: cols [k*196 + j*14, ... +14); dst on partition j
                        g.indirect_dma_start(
                            out=stage[j:j + 1, :].rearrange("o f -> o f ()"),
                            out_offset=None, in_=rtab[:],
                            in_offset=IndirectOffsetOnAxis(
                                ap=esb[:, k * 196 + j * 14:k * 196 + (j + 1) * 14], axis=0),
                        ).then_inc(gsem, 16)
                        gc[0] += 16
                    g.wait_ge(gsem, gc[0])
                    # reshape [NSPLIT, SPLITC] -> [128, 196] row-major via DRAM bounce
                    dst = ru if which == 0 else rv
                    g.dma_start(out=stg_d[:].rearrange("(p f) -> p f", p=NSPLIT), in_=stage[:]).then_inc(dsem, 16)
                    dc[0] += 16
                    g.wait_ge(dsem, dc[0])
                    g.dma_start(out=dst[:], in_=stg_d[:].rearrange("(p f) -> p f", p=128)
                                ).then_inc(dsem, 16)
                    dc[0] += 16
                    g.wait_ge(dsem, dc[0])
                # load ew chunk (flat [128, 196] row-major)
                dma(ewc[:], ew_in[k * ECH:(k + 1) * ECH].rearrange("(p f) -> p f", p=128))
                g.wait_ge(dsem, dc[0])
                g.sem_inc(qsem, 1)            # chunk k ready for vector
                g.wait_ge(vsem, k + 1)        # vector done with chunk k
                dma(out_newu[k * ECH:(k + 1) * ECH].rearrange("(p f) -> p f", p=128), nu_i[:])
                dma(out_newv[k * ECH:(k + 1) * ECH].rearrange("(p f) -> p f", p=128), nv_i[:])
                dma(out_mask[k * ECH:(k + 1) * ECH].rearrange("(p f) -> p f", p=128), mk8[:])
                dma(out_ew[k * ECH:(k + 1) * ECH].rearrange("(p f) -> p f", p=128), ewo[:])
                g.wait_ge(dsem, dc[0])

        @block.vector
        def _(v):
            for k in range(NCHUNK):
                v.wait_ge(qsem, k + 1)
                # mu = ru < K ; mv = rv < K
                v.tensor_scalar(out=mu[:], in0=ru[:], scalar1=float(K), scalar2=None, op0=Alu.is_lt)
                v.tensor_scalar(out=mv[:], in0=rv[:], scalar1=float(K), scalar2=None, op0=Alu.is_lt)
                # newu = mu*(ru+1) - 1
                v.tensor_scalar(out=t0[:], in0=ru[:], scalar1=1.0, scalar2=None, op0=Alu.add)
                v.tensor_tensor(out=t0[:], in0=t0[:], in1=mu[:], op=Alu.mult)
                v.tensor_scalar(out=t0[:], in0=t0[:], scalar1=1.0, scalar2=None, op0=Alu.subtract)
                v.tensor_copy(out=nu_i[:], in_=t0[:])
                v.tensor_scalar(out=t0[:], in0=rv[:], scalar1=1.0, scalar2=None, op0=Alu.add)
                v.tensor_tensor(out=t0[:], in0=t0[:], in1=mv[:], op=Alu.mult)
                v.tensor_scalar(out=t0[:], in0=t0[:], scalar1=1.0, scalar2=None, op0=Alu.subtract)
                v.tensor_copy(out=nv_i[:], in_=t0[:])
                # mask = mu*mv ; ew = w*mask
                v.tensor_tensor(out=t0[:], in0=mu[:], in1=mv[:], op=Alu.mult)
                v.tensor_copy(out=mk8[:], in_=t0[:])
                v.tensor_tensor(out=ewo[:], in0=ewc[:], in1=t0[:], op=Alu.mult)
                v.sem_inc(vsem, 1)

    nc.compile()
    return nc


def kernel(x, edge_index, edge_weight, batch, W1, b1, W2, b2):
    _install_profile_hook()
    from concourse.bass_utils import run_bass_kernel_spmd

    x = np.ascontiguousarray(np.asarray(x, np.float32))
    edge_index = np.asarray(edge_index)
    edge_weight = np.asarray(edge_weight, np.float32)
    batch = np.asarray(batch)
    W1 = np.asarray(W1, np.float32)
    b1 = np.asarray(b1, np.float32)
    W2 = np.asarray(W2, np.float32)
    b2 = np.asarray(b2, np.float32)

    # ---------------- NEFF-1: scores ----------------
    if "scores" not in _BUILT:
        _BUILT["scores"] = build_scores()
    nc1 = _BUILT["scores"]

    x_pad = np.zeros((NPAD, D), np.float32)
    x_pad[:N] = x
    w1t_np = np.ascontiguousarray(W1.T)                 # [256, 64]
    b1_np = np.ascontiguousarray(b1.reshape(H, 1))
    w2t_np = np.ascontiguousarray(W2.reshape(1, H).T)   # [64, 1]
    b2_np = np.ascontiguousarray(b2.reshape(1, 1))
    ident = np.eye(128, dtype=np.float32)

    in1 = []
    for c in range(NCORES):
        in1.append({
            "x_sh": x_pad[c * SH:(c + 1) * SH],
            "w1t": w1t_np, "b1_in": b1_np, "w2t": w2t_np, "b2_in": b2_np,
            "ident": ident,
        })
    import os
    _tr = os.environ.get("KTRACE", "0") == "1"
    res1 = run_bass_kernel_spmd(nc1, in1, core_ids=list(range(NCORES)), trace=_tr)
    # scores: out_s [128, NT] wrapped: node c*SH + t*128 + p at [p, t]
    scores = np.empty(NPAD, np.float32)
    for c in range(NCORES):
        sw = np.asarray(res1.results[c]["out_s"])       # [128, NT]
        scores[c * SH:(c + 1) * SH] = sw.T.reshape(-1)  # i = t*128 + p
    scores[:N] = (scores[:N] + b2.reshape(1)[0]).astype(np.float32)
    ktime1 = getattr(res1, "exec_time_ns", None)

    # ---------------- host: rank tables ----------------
    bits = scores.view(np.int32).astype(np.int64)
    key = (bits ^ ((bits >> 31) | np.int64(-0x80000000))) & 0xFFFFFFFF
    gid = np.arange(NPAD, dtype=np.int64)
    key = np.where(gid < N, key, gid - N)     # pads: distinct tiny keys
    order = np.lexsort((gid, -key))           # descending key, ties by id
    rank = np.empty(NPAD, np.int64)
    rank[order] = gid
    inv = order                               # rank -> node id

    # ---------------- NEFF-2: gathers ----------------
    if "main" not in _BUILT:
        _BUILT["main"] = build_main()
    nc2 = _BUILT["main"]

    ei = edge_index.astype(np.int64)
    eu_flat = np.zeros(EPAD, np.int32)
    ev_flat = np.zeros(EPAD, np.int32)
    ew_flat = np.zeros(EPAD, np.float32)
    eu_flat[:E] = ei[0]
    ev_flat[:E] = ei[1]
    ew_flat[:E] = edge_weight
    batch_pad = np.zeros((NPAD, 1), np.float32)
    batch_pad[:N, 0] = batch.astype(np.float32)
    rtab_np = rank.astype(np.float32).reshape(NPAD, 1)

    in2 = []
    for c in range(NCORES):
        sl = slice(c * ESH, (c + 1) * ESH)
        eu_sh = eu_flat[sl].reshape(ESH // 128, 128).T.copy()   # wrapped
        ev_sh = ev_flat[sl].reshape(ESH // 128, 128).T.copy()
        # ew must be chunk-row-major: edge (k*ECH + p*196 + f) at [p, k*196+f]
        myidx = inv[c * 6250: c * 6250 + KS].astype(np.int32)
        myidx_w = myidx.reshape(KS // 128, 128).T.copy()
        in2.append({
            "x_full": x_pad, "rtab": rtab_np, "batch_all": batch_pad,
            "eu_in": eu_sh, "ev_in": ev_sh, "ew_in": ew_flat[sl].copy(),
            "myidx_in": myidx_w,
        })
    res2 = run_bass_kernel_spmd(nc2, in2, core_ids=list(range(NCORES)), trace=_tr)
    ktime2 = getattr(res2, "exec_time_ns", None)
    kernel.exec_times = (ktime1, ktime2)

    # ---------------- assemble ----------------
    new_ei = np.empty((2, E), np.int32)
    mask = np.empty(E, bool)
    ew_out = np.empty(E, np.float32)
    x_kept = np.empty((K, D), np.float32)
    batch_kept = np.empty(K, batch.dtype)
    for c in range(NCORES):
        r = res2.results[c]
        lo, hi = c * ESH, min((c + 1) * ESH, E)
        n = hi - lo
        if n > 0:
            new_ei[0, lo:hi] = np.asarray(r["out_newu"])[:n]
            new_ei[1, lo:hi] = np.asarray(r["out_newv"])[:n]
            mask[lo:hi] = np.asarray(r["out_mask"])[:n] != 0
            ew_out[lo:hi] = np.asarray(r["out_ew"])[:n]
        # kept rows: out_xk row j = rank c*6250 + j
        klo = c * 6250
        kn = min(6250, K - klo)
        x_kept[klo:klo + kn] = np.asarray(r["out_xk"])[:kn]
        batch_kept[klo:klo + kn] = np.asarray(r["out_bk"])[0, :kn].astype(batch.dtype)
    indices = inv[:K].astype(np.int32)
    return x_kept, new_ei, ew_out, mask, batch_kept, indices


# revision 8
# speedup vs baseline: 1.1635x; 1.1635x over previous
"""AdaptiveGraphPooling on 8 TRN2 NeuronCores.

kernel(**inputs) -> (x_kept, new_ei, ew, mask, batch_kept, indices)

Device NEFF-1: per-node-shard attention scores (PE transpose + matmuls + relu).
Host:          monotone-key argsort of the 100k scores (rank tables).
Device NEFF-2: all heavy data movement — 6.4M-element edge remap gathers of the
               rank table, mask/ew computation, 50k x-row gathers (x_kept),
               batch gathers — sharded across the 8 cores.
"""
import numpy as np

N = 100000
D = 256
H = 64
E = 3200000
K = 50000
NCORES = 8
NPAD = 100352            # 8 * 12544
SH = NPAD // NCORES      # 12544 nodes/core
NT = SH // 128           # 98 node tiles/core
EPAD = 3211264           # 8 * 401408 padded edges
ESH = EPAD // NCORES     # 401408 edges/core
EF = ESH // 128          # 3136
ECH = 25088              # edges per gather chunk (196 cols)
NCHUNK = ESH // ECH      # 16
NSPLIT = 14              # indirect-gather instruction split per chunk
SPLITC = ECH // NSPLIT   # 1792 descs per instruction
KS = 6272                # kept rows handled per core (first 6250 used)

_BUILT = {}


def _install_profile_hook():
    import sys, types
    import antenv
    try:
        from antenv.axon_hooks import get_axon_ntff_profile_hook
        if get_axon_ntff_profile_hook() is not None:
            return
    except ImportError:
        mod = types.ModuleType("antenv.axon_hooks")
        mod._hook = None
        def _set(hook):
            mod._hook = hook
        def _get():
            return mod._hook
        mod.set_axon_ntff_profile_hook = _set
        mod.get_axon_ntff_profile_hook = _get
        sys.modules["antenv.axon_hooks"] = mod
        antenv.axon_hooks = mod
    try:
        from antenv.axon_hooks import set_axon_ntff_profile_hook
        from trn_agent_boot.trn_boot import _ntff_profile_via_ctypes
        set_axon_ntff_profile_hook(_ntff_profile_via_ctypes('/opt/axon/libaxon_pjrt.so'))
    except Exception:
        pass


def build_scores():
    """NEFF-1: scores[i] = W2 @ relu(W1 @ x_i + b1) + b2 for the core's shard."""
    import concourse.bass as bass
    import concourse.bacc as bacc
    import concourse.mybir as mybir
    f32 = mybir.dt.float32

    nc = bacc.Bacc("TRN2", debug=False)
    x_sh = nc.declare_dram_parameter("x_sh", [SH, D], f32, isOutput=False)
    w1t = nc.declare_dram_parameter("w1t", [D, H], f32, isOutput=False)
    b1_in = nc.declare_dram_parameter("b1_in", [H, 1], f32, isOutput=False)
    w2t = nc.declare_dram_parameter("w2t", [H, 1], f32, isOutput=False)
    b2_in = nc.declare_dram_parameter("b2_in", [1, 1], f32, isOutput=False)
    ident_in = nc.declare_dram_parameter("ident", [128, 128], f32, isOutput=False)
    out_s = nc.declare_dram_parameter("out_s", [128, NT], f32, isOutput=True)

    with (
        nc.Block() as block,
        nc.sbuf_tensor("xa", [128, 256], f32) as xa,
        nc.sbuf_tensor("xb", [128, 256], f32) as xb,
        nc.sbuf_tensor("w1t_sb", [128, 2 * H], f32) as w1t_sb,
        nc.sbuf_tensor("b1_sb", [H, 1], f32) as b1_sb,
        nc.sbuf_tensor("w2t_sb", [H, 1], f32) as w2t_sb,
        nc.sbuf_tensor("b2_sb", [1, 1], f32) as b2_sb,
        nc.sbuf_tensor("id_sb", [128, 128], f32) as id_sb,
        nc.sbuf_tensor("xT_sb", [128, 256], f32) as xT_sb,
        nc.sbuf_tensor("hT_sb", [H, 128], f32) as hT_sb,
        nc.sbuf_tensor("sw", [128, NT], f32) as sw,
        nc.psum_tensor("ps_t", [128, 256], f32) as ps_t,
        nc.psum_tensor("ps_h", [H, 128], f32) as ps_h,
        nc.psum_tensor("ps_s", [128, 1], f32) as ps_s,
        nc.semaphore("ld") as ld,       # x tile loads (sync engine), +16 each
        nc.semaphore("pe") as pe,       # tensor engine progress, +1 steps
        nc.semaphore("dv") as dv,       # vector progress
        nc.semaphore("sc") as sc,       # scalar progress
        nc.semaphore("done") as done,
    ):
        @block.sync
        def _(s):
            s.dma_start(out=w1t_sb[:, 0:H], in_=w1t[0:128, :]).then_inc(ld, 16)
            s.dma_start(out=w1t_sb[:, H:2 * H], in_=w1t[128:256, :]).then_inc(ld, 16)
            s.dma_start(out=b1_sb[:], in_=b1_in[:]).then_inc(ld, 16)
            s.dma_start(out=w2t_sb[:], in_=w2t[:]).then_inc(ld, 16)
            s.dma_start(out=b2_sb[:], in_=b2_in[:]).then_inc(ld, 16)
            s.dma_start(out=id_sb[:], in_=ident_in[:]).then_inc(ld, 16)
            for t in range(NT):
                buf = xa if t % 2 == 0 else xb
                if t >= 2:
                    # wait until PE consumed tile t-2 (transpose done => step 1 of t-2)
                    s.wait_ge(pe, (t - 2) * 4 + 2)
                s.dma_start(out=buf[:], in_=x_sh[t * 128:(t + 1) * 128, :]).then_inc(ld, 16)
            s.wait_ge(dv, 3 * NT)
            s.dma_start(out=out_s[:], in_=sw[:]).then_inc(done, 16)

        @block.tensor
        def _(te):
            for t in range(NT):
                buf = xa if t % 2 == 0 else xb
                te.wait_ge(ld, 96 + (t + 1) * 16)
                if t > 0:
                    te.wait_ge(dv, 3 * (t - 1) + 2)   # ps_t free (xT copies of t-1 done)
                te.transpose(out=ps_t[:, 0:128], in_=buf[:, 0:128], identity=id_sb[:]).then_inc(pe, 1)
                te.transpose(out=ps_t[:, 128:256], in_=buf[:, 128:256], identity=id_sb[:]).then_inc(pe, 1)
                te.wait_ge(dv, t * 3 + 2)             # xT of t ready
                te.matmul(out=ps_h[:], lhsT=w1t_sb[:, 0:H], rhs=xT_sb[:, 0:128], start=True, stop=False)
                te.matmul(out=ps_h[:], lhsT=w1t_sb[:, H:2 * H], rhs=xT_sb[:, 128:256],
                          start=False, stop=True).then_inc(pe, 1)
                te.wait_ge(sc, t + 1)                 # relu of t done
                if t > 0:
                    te.wait_ge(dv, 3 * (t - 1) + 3)   # ps_s free
                te.matmul(out=ps_s[:], lhsT=hT_sb[:], rhs=w2t_sb[:], start=True, stop=True).then_inc(pe, 1)

        @block.vector
        def _(v):
            for t in range(NT):
                v.wait_ge(pe, t * 4 + 2)
                v.tensor_copy(out=xT_sb[:, 0:128], in_=ps_t[:, 0:128]).then_inc(dv, 1)
                v.tensor_copy(out=xT_sb[:, 128:256], in_=ps_t[:, 128:256]).then_inc(dv, 1)
                v.wait_ge(pe, t * 4 + 4)
                v.tensor_copy(out=sw[:, t:t + 1], in_=ps_s[:]).then_inc(dv, 1)

        @block.scalar
        def _(sce):
            import concourse.mybir as mybir
            for t in range(NT):
                sce.wait_ge(pe, t * 4 + 3)
                sce.activation(out=hT_sb[:], in_=ps_h[:],
                               func=mybir.ActivationFunctionType.Relu,
                               bias=b1_sb[:], scale=1.0).then_inc(sc, 1)

    nc.compile()
    return nc


def build_main():
    """NEFF-2: edge remap + mask/ew + x_kept/batch gathers."""
    import concourse.bass as bass
    import concourse.bacc as bacc
    import concourse.mybir as mybir
    from concourse.bass import IndirectOffsetOnAxis
    f32 = mybir.dt.float32
    i32 = mybir.dt.int32
    u8 = mybir.dt.uint8
    Alu = mybir.AluOpType

    nc = bacc.Bacc("TRN2", debug=False)
    x_full = nc.declare_dram_parameter("x_full", [NPAD, D], f32, isOutput=False)
    rtab = nc.declare_dram_parameter("rtab", [NPAD, 1], f32, isOutput=False)  # rank as f32
    batch_all = nc.declare_dram_parameter("batch_all", [NPAD, 1], f32, isOutput=False)
    eu_in = nc.declare_dram_parameter("eu_in", [128, EF], i32, isOutput=False)   # wrapped
    ev_in = nc.declare_dram_parameter("ev_in", [128, EF], i32, isOutput=False)   # wrapped
    ew_in = nc.declare_dram_parameter("ew_in", [ESH], f32, isOutput=False)       # flat
    myidx_in = nc.declare_dram_parameter("myidx_in", [128, KS // 128], i32, isOutput=False)  # wrapped node ids

    out_newu = nc.declare_dram_parameter("out_newu", [ESH], i32, isOutput=True)
    out_newv = nc.declare_dram_parameter("out_newv", [ESH], i32, isOutput=True)
    out_mask = nc.declare_dram_parameter("out_mask", [ESH], u8, isOutput=True)
    out_ew = nc.declare_dram_parameter("out_ew", [ESH], f32, isOutput=True)
    out_xk = nc.declare_dram_parameter("out_xk", [KS, D], f32, isOutput=True)
    out_bk = nc.declare_dram_parameter("out_bk", [1, KS], f32, isOutput=True)

    KC = KS // 128   # 49
    stg_d = nc.dram_tensor("stg_d", [ECH], f32)

    with (
        nc.Block() as block,
        nc.sbuf_tensor("stage", [1, ECH], f32) as stage,
        nc.sbuf_tensor("eu_sb", [128, EF], i32) as eu_sb,
        nc.sbuf_tensor("ev_sb", [128, EF], i32) as ev_sb,
        nc.sbuf_tensor("ru", [128, 196], f32) as ru,
        nc.sbuf_tensor("rv", [128, 196], f32) as rv,
        nc.sbuf_tensor("mu", [128, 196], f32) as mu,
        nc.sbuf_tensor("mv", [128, 196], f32) as mv,
        nc.sbuf_tensor("t0", [128, 196], f32) as t0,
        nc.sbuf_tensor("nu_i", [128, 196], i32) as nu_i,
        nc.sbuf_tensor("nv_i", [128, 196], i32) as nv_i,
        nc.sbuf_tensor("mk8", [128, 196], u8) as mk8,
        nc.sbuf_tensor("ewc", [128, 196], f32) as ewc,
        nc.sbuf_tensor("ewo", [128, 196], f32) as ewo,
        nc.sbuf_tensor("myidx_sb", [128, KC], i32) as myidx_sb,
        nc.sbuf_tensor("xg", [128, KC, D], f32) as xg,
        nc.sbuf_tensor("bk_sb", [1, KS], f32) as bk_sb,
        nc.semaphore("dsem") as dsem,
        nc.semaphore("gsem") as gsem,
        nc.semaphore("vsem") as vsem,
        nc.semaphore("qsem") as qsem,
    ):
        dc = [0]
        gc = [0]

        @block.gpsimd
        def _(g):
            def dma(out, in_):
                g.dma_start(out=out, in_=in_).then_inc(dsem, 16)
                dc[0] += 16

            dma(eu_sb[:], eu_in[:])
            dma(ev_sb[:], ev_in[:])
            dma(myidx_sb[:], myidx_in[:])
            g.wait_ge(dsem, dc[0])

            # ---- x_kept: 49 row-gather instructions ----
            for j in range(KC):
                g.indirect_dma_start(
                    out=xg[:, j, :], out_offset=None, in_=x_full[:],
                    in_offset=IndirectOffsetOnAxis(ap=myidx_sb[:, j:j + 1], axis=0),
                ).then_inc(gsem, 16)
                gc[0] += 16
            # batch_kept: one [1, KS] gather
            g.indirect_dma_start(
                out=bk_sb[:].rearrange("o f -> o f ()"), out_offset=None,
                in_=batch_all[:],
                in_offset=IndirectOffsetOnAxis(ap=myidx_sb[:, :], axis=0),
            ).then_inc(gsem, 16)
            gc[0] += 16
            g.wait_ge(gsem, gc[0])
            dma(out_bk[:], bk_sb[:])
            dma(out_xk[:].rearrange("(f p) d -> p f d", p=128), xg[:])
            g.wait_ge(dsem, dc[0])

            # ---- edges ----
            for k in range(NCHUNK):
                for which, esb in ((0, eu_sb), (1, ev_sb)):
                    for j in range(NSPLIT):
                        # stream slice: cols [k*196 + j*14, ... +14)
                        g.indirect_dma_start(
                            out=stage[:, j * SPLITC:(j + 1) * SPLITC].rearrange("o f -> o f ()"),
                            out_offset=None, in_=rtab[:],
                            in_offset=IndirectOffsetOnAxis(
                                ap=esb[:, k * 196 + j * 14:k * 196 + (j + 1) * 14], axis=0),
                        ).then_inc(gsem, 16)
                        gc[0] += 16
                    g.wait_ge(gsem, gc[0])
                    # reshape [1, ECH] -> [128, 196] row-major via DRAM bounce
                    dst = ru if which == 0 else rv
                    g.dma_start(out=stg_d[None, :], in_=stage[:]).then_inc(dsem, 16)
                    dc[0] += 16
                    g.wait_ge(dsem, dc[0])
                    g.dma_start(out=dst[:], in_=stg_d[:].rearrange("(p f) -> p f", p=128)
                                ).then_inc(dsem, 16)
                    dc[0] += 16
                    g.wait_ge(dsem, dc[0])
                # load ew chunk (flat [128, 196] row-major)
                dma(ewc[:], ew_in[k * ECH:(k + 1) * ECH].rearrange("(p f) -> p f", p=128))
                g.wait_ge(dsem, dc[0])
                g.sem_inc(qsem, 1)            # chunk k ready for vector
                g.wait_ge(vsem, k + 1)        # vector done with chunk k
                dma(out_newu[k * ECH:(k + 1) * ECH].rearrange("(p f) -> p f", p=128), nu_i[:])
                dma(out_newv[k * ECH:(k + 1) * ECH].rearrange("(p f) -> p f", p=128), nv_i[:])
                dma(out_mask[k * ECH:(k + 1) * ECH].rearrange("(p f) -> p f", p=128), mk8[:])
                dma(out_ew[k * ECH:(k + 1) * ECH].rearrange("(p f) -> p f", p=128), ewo[:])
                g.wait_ge(dsem, dc[0])

        @block.vector
        def _(v):
            for k in range(NCHUNK):
                v.wait_ge(qsem, k + 1)
                # mu = ru < K ; mv = rv < K
                v.tensor_scalar(out=mu[:], in0=ru[:], scalar1=float(K), scalar2=None, op0=Alu.is_lt)
                v.tensor_scalar(out=mv[:], in0=rv[:], scalar1=float(K), scalar2=None, op0=Alu.is_lt)
                # newu = mu*(ru+1) - 1
                v.tensor_scalar(out=t0[:], in0=ru[:], scalar1=1.0, scalar2=None, op0=Alu.add)
                v.tensor_tensor(out=t0[:], in0=t0[:], in1=mu[:], op=Alu.mult)
                v.tensor_scalar(out=t0[:], in0=t0[:], scalar1=1.0, scalar2=None, op0=Alu.subtract)
                v.tensor_copy(out=nu_i[:], in_=t0[:])
                v.tensor_scalar(out=t0[:], in0=rv[:], scalar1=1.0, scalar2=None, op0=Alu.add)
                v.tensor_tensor(out=t0[:], in0=t0[:], in1=mv[:], op=Alu.mult)
                v.tensor_scalar(out=t0[:], in0=t0[:], scalar1=1.0, scalar2=None, op0=Alu.subtract)
                v.tensor_copy(out=nv_i[:], in_=t0[:])
                # mask = mu*mv ; ew = w*mask
                v.tensor_tensor(out=t0[:], in0=mu[:], in1=mv[:], op=Alu.mult)
                v.tensor_copy(out=mk8[:], in_=t0[:])
                v.tensor_tensor(out=ewo[:], in0=ewc[:], in1=t0[:], op=Alu.mult)
                v.sem_inc(vsem, 1)

    nc.compile()
    return nc


def kernel(x, edge_index, edge_weight, batch, W1, b1, W2, b2):
    _install_profile_hook()
    from concourse.bass_utils import run_bass_kernel_spmd

    x = np.ascontiguousarray(np.asarray(x, np.float32))
    edge_index = np.asarray(edge_index)
    edge_weight = np.asarray(edge_weight, np.float32)
    batch = np.asarray(batch)
    W1 = np.asarray(W1, np.float32)
    b1 = np.asarray(b1, np.float32)
    W2 = np.asarray(W2, np.float32)
    b2 = np.asarray(b2, np.float32)

    # ---------------- NEFF-1: scores ----------------
    if "scores" not in _BUILT:
        _BUILT["scores"] = build_scores()
    nc1 = _BUILT["scores"]

    x_pad = np.zeros((NPAD, D), np.float32)
    x_pad[:N] = x
    w1t_np = np.ascontiguousarray(W1.T)                 # [256, 64]
    b1_np = np.ascontiguousarray(b1.reshape(H, 1))
    w2t_np = np.ascontiguousarray(W2.reshape(1, H).T)   # [64, 1]
    b2_np = np.ascontiguousarray(b2.reshape(1, 1))
    ident = np.eye(128, dtype=np.float32)

    in1 = []
    for c in range(NCORES):
        in1.append({
            "x_sh": x_pad[c * SH:(c + 1) * SH],
            "w1t": w1t_np, "b1_in": b1_np, "w2t": w2t_np, "b2_in": b2_np,
            "ident": ident,
        })
    import os
    _tr = os.environ.get("KTRACE", "0") == "1"
    res1 = run_bass_kernel_spmd(nc1, in1, core_ids=list(range(NCORES)), trace=_tr)
    # scores: out_s [128, NT] wrapped: node c*SH + t*128 + p at [p, t]
    scores = np.empty(NPAD, np.float32)
    for c in range(NCORES):
        sw = np.asarray(res1.results[c]["out_s"])       # [128, NT]
        scores[c * SH:(c + 1) * SH] = sw.T.reshape(-1)  # i = t*128 + p
    scores[:N] = (scores[:N] + b2.reshape(1)[0]).astype(np.float32)
    ktime1 = getattr(res1, "exec_time_ns", None)

    # ---------------- host: rank tables ----------------
    bits = scores.view(np.int32).astype(np.int64)
    key = (bits ^ ((bits >> 31) | np.int64(-0x80000000))) & 0xFFFFFFFF
    gid = np.arange(NPAD, dtype=np.int64)
    key = np.where(gid < N, key, gid - N)     # pads: distinct tiny keys
    order = np.lexsort((gid, -key))           # descending key, ties by id
    rank = np.empty(NPAD, np.int64)
    rank[order] = gid
    inv = order                               # rank -> node id

    # ---------------- NEFF-2: gathers ----------------
    if "main" not in _BUILT:
        _BUILT["main"] = build_main()
    nc2 = _BUILT["main"]

    ei = edge_index.astype(np.int64)
    eu_flat = np.zeros(EPAD, np.int32)
    ev_flat = np.zeros(EPAD, np.int32)
    ew_flat = np.zeros(EPAD, np.float32)
    eu_flat[:E] = ei[0]
    ev_flat[:E] = ei[1]
    ew_flat[:E] = edge_weight
    batch_pad = np.zeros((NPAD, 1), np.float32)
    batch_pad[:N, 0] = batch.astype(np.float32)
    rtab_np = rank.astype(np.float32).reshape(NPAD, 1)

    in2 = []
    for c in range(NCORES):
        sl = slice(c * ESH, (c + 1) * ESH)
        eu_sh = eu_flat[sl].reshape(ESH // 128, 128).T.copy()   # wrapped
        ev_sh = ev_flat[sl].reshape(ESH // 128, 128).T.copy()
        # ew must be chunk-row-major: edge (k*ECH + p*196 + f) at [p, k*196+f]
        myidx = inv[c * 6250: c * 6250 + KS].astype(np.int32)
        myidx_w = myidx.reshape(KS // 128, 128).T.copy()
        in2.append({
            "x_full": x_pad, "rtab": rtab_np, "batch_all": batch_pad,
            "eu_in": eu_sh, "ev_in": ev_sh, "ew_in": ew_flat[sl].copy(),
            "myidx_in": myidx_w,
        })
    res2 = run_bass_kernel_spmd(nc2, in2, core_ids=list(range(NCORES)), trace=_tr)
    ktime2 = getattr(res2, "exec_time_ns", None)
    kernel.exec_times = (ktime1, ktime2)

    # ---------------- assemble ----------------
    new_ei = np.empty((2, E), np.int32)
    mask = np.empty(E, bool)
    ew_out = np.empty(E, np.float32)
    x_kept = np.empty((K, D), np.float32)
    batch_kept = np.empty(K, batch.dtype)
    for c in range(NCORES):
        r = res2.results[c]
        lo, hi = c * ESH, min((c + 1) * ESH, E)
        n = hi - lo
        if n > 0:
            new_ei[0, lo:hi] = np.asarray(r["out_newu"])[:n]
            new_ei[1, lo:hi] = np.asarray(r["out_newv"])[:n]
            mask[lo:hi] = np.asarray(r["out_mask"])[:n] != 0
            ew_out[lo:hi] = np.asarray(r["out_ew"])[:n]
        # kept rows: out_xk row j = rank c*6250 + j
        klo = c * 6250
        kn = min(6250, K - klo)
        x_kept[klo:klo + kn] = np.asarray(r["out_xk"])[:kn]
        batch_kept[klo:klo + kn] = np.asarray(r["out_bk"])[0, :kn].astype(batch.dtype)
    indices = inv[:K].astype(np.int32)
    return x_kept, new_ei, ew_out, mask, batch_kept, indices


# revision 12
# speedup vs baseline: 1.2725x; 1.0936x over previous
"""AdaptiveGraphPooling on 8 TRN2 NeuronCores.

kernel(**inputs) -> (x_kept, new_ei, ew, mask, batch_kept, indices)

Device NEFF-1: per-node-shard attention scores (PE transpose + matmuls + relu).
Host:          monotone-key argsort of the 100k scores (rank tables).
Device NEFF-2: all heavy data movement — 6.4M-element edge remap gathers of the
               rank table, mask/ew computation, 50k x-row gathers (x_kept),
               batch gathers — sharded across the 8 cores.
"""
import numpy as np

N = 100000
D = 256
H = 64
E = 3200000
K = 50000
NCORES = 8
NPAD = 100352            # 8 * 12544
SH = NPAD // NCORES      # 12544 nodes/core
NT = SH // 128           # 98 node tiles/core
EPAD = 3211264           # 8 * 401408 padded edges
ESH = EPAD // NCORES     # 401408 edges/core
EF = ESH // 128          # 3136
ECH = 25088              # edges per gather chunk (196 cols)
NCHUNK = ESH // ECH      # 16
NSPLIT = 14              # indirect-gather instruction split per chunk
SPLITC = ECH // NSPLIT   # 1792 descs per instruction
KS = 6272                # kept rows handled per core (first 6250 used)

_BUILT = {}


def _install_profile_hook():
    import sys, types
    import antenv
    try:
        from antenv.axon_hooks import get_axon_ntff_profile_hook
        if get_axon_ntff_profile_hook() is not None:
            return
    except ImportError:
        mod = types.ModuleType("antenv.axon_hooks")
        mod._hook = None
        def _set(hook):
            mod._hook = hook
        def _get():
            return mod._hook
        mod.set_axon_ntff_profile_hook = _set
        mod.get_axon_ntff_profile_hook = _get
        sys.modules["antenv.axon_hooks"] = mod
        antenv.axon_hooks = mod
    try:
        from antenv.axon_hooks import set_axon_ntff_profile_hook
        from trn_agent_boot.trn_boot import _ntff_profile_via_ctypes
        set_axon_ntff_profile_hook(_ntff_profile_via_ctypes('/opt/axon/libaxon_pjrt.so'))
    except Exception:
        pass


def build_scores():
    """NEFF-1: scores[i] = W2 @ relu(W1 @ x_i + b1) for the core's shard.

    Grouped pipeline: G=7 node tiles (896 nodes) per group, 14 groups.
    """
    import concourse.bass as bass
    import concourse.bacc as bacc
    import concourse.mybir as mybir
    f32 = mybir.dt.float32
    G = 7
    NG = NT // G                      # 14 groups
    GW = G * 256                      # 1792 cols of x per group

    nc = bacc.Bacc("TRN2", debug=False)
    x_sh = nc.declare_dram_parameter("x_sh", [SH, D], f32, isOutput=False)
    w1t = nc.declare_dram_parameter("w1t", [D, H], f32, isOutput=False)
    b1_in = nc.declare_dram_parameter("b1_in", [H, 1], f32, isOutput=False)
    w2t = nc.declare_dram_parameter("w2t", [H, 1], f32, isOutput=False)
    b2_in = nc.declare_dram_parameter("b2_in", [1, 1], f32, isOutput=False)
    ident_in = nc.declare_dram_parameter("ident", [128, 128], f32, isOutput=False)
    out_s = nc.declare_dram_parameter("out_s", [128, NT], f32, isOutput=True)

    with (
        nc.Block() as block,
        nc.sbuf_tensor("xbuf", [128, 2, G, 256], f32) as xbuf,
        nc.sbuf_tensor("w1t_sb", [128, 2 * H], f32) as w1t_sb,
        nc.sbuf_tensor("b1_sb", [H, 1], f32) as b1_sb,
        nc.sbuf_tensor("w2t_sb", [H, 1], f32) as w2t_sb,
        nc.sbuf_tensor("b2_sb", [1, 1], f32) as b2_sb,
        nc.sbuf_tensor("id_sb", [128, 128], f32) as id_sb,
        nc.sbuf_tensor("xT_sb", [128, 2, GW], f32) as xT_sb,
        nc.sbuf_tensor("hT_sb", [H, 2, G * 128], f32) as hT_sb,
        nc.sbuf_tensor("sw", [128, NT], f32) as sw,
        nc.psum_tensor("ps_t", [128, GW], f32) as ps_t,
        nc.psum_tensor("ps_h", [H, G * 128], f32) as ps_h,
        nc.psum_tensor("ps_s", [128, G], f32) as ps_s,
        nc.semaphore("ld") as ld,
        nc.semaphore("pe") as pe,
        nc.semaphore("dv") as dv,
        nc.semaphore("sc") as sc,
        nc.semaphore("done") as done,
    ):
        @block.sync
        def _(s):
            s.dma_start(out=w1t_sb[:, 0:H], in_=w1t[0:128, :]).then_inc(ld, 16)
            s.dma_start(out=w1t_sb[:, H:2 * H], in_=w1t[128:256, :]).then_inc(ld, 16)
            s.dma_start(out=b1_sb[:], in_=b1_in[:]).then_inc(ld, 16)
            s.dma_start(out=w2t_sb[:], in_=w2t[:]).then_inc(ld, 16)
            s.dma_start(out=b2_sb[:], in_=b2_in[:]).then_inc(ld, 16)
            s.dma_start(out=id_sb[:], in_=ident_in[:]).then_inc(ld, 16)
            for g in range(NG):
                if g >= 2:
                    s.wait_ge(pe, 3 * (g - 2) + 1)   # xbuf[g%2] consumed
                s.dma_start(
                    out=xbuf[:, g % 2],
                    in_=x_sh[g * 128 * G:(g + 1) * 128 * G, :].rearrange(
                        "(t p) d -> p t d", p=128),
                ).then_inc(ld, 16)
            s.wait_ge(dv, 2 * NG)
            s.dma_start(out=out_s[:], in_=sw[:]).then_inc(done, 16)

        @block.tensor
        def _(te):
            for g in range(NG):
                te.wait_ge(ld, 96 + (g + 1) * 16)
                if g > 0:
                    te.wait_ge(dv, 2 * (g - 1) + 1)   # ps_t free
                for t in range(G):
                    te.transpose(out=ps_t[:, t * 256:t * 256 + 128],
                                 in_=xbuf[:, g % 2, t, 0:128], identity=id_sb[:])
                    i2 = te.transpose(out=ps_t[:, t * 256 + 128:(t + 1) * 256],
                                      in_=xbuf[:, g % 2, t, 128:256], identity=id_sb[:])
                i2.then_inc(pe, 1)
                te.wait_ge(dv, 2 * g + 1)             # xT[g%2] ready
                for t in range(G):
                    te.matmul(out=ps_h[:, t * 128:(t + 1) * 128],
                              lhsT=w1t_sb[:, 0:H],
                              rhs=xT_sb[:, g % 2, t * 256:t * 256 + 128],
                              start=True, stop=False)
                    i3 = te.matmul(out=ps_h[:, t * 128:(t + 1) * 128],
                                   lhsT=w1t_sb[:, H:2 * H],
                                   rhs=xT_sb[:, g % 2, t * 256 + 128:(t + 1) * 256],
                                   start=False, stop=True)
                i3.then_inc(pe, 1)
                te.wait_ge(sc, g + 1)                 # relu done
                if g > 0:
                    te.wait_ge(dv, 2 * g)             # ps_s free
                for t in range(G):
                    i4 = te.matmul(out=ps_s[:, t:t + 1],
                                   lhsT=hT_sb[:, g % 2, t * 128:(t + 1) * 128],
                                   rhs=w2t_sb[:], start=True, stop=True)
                i4.then_inc(pe, 1)

        @block.vector
        def _(v):
            for g in range(NG):
                v.wait_ge(pe, 3 * g + 1)
                v.tensor_copy(out=xT_sb[:, g % 2], in_=ps_t[:]).then_inc(dv, 1)
                v.wait_ge(pe, 3 * g + 3)
                v.tensor_copy(out=sw[:, g * G:(g + 1) * G], in_=ps_s[:]).then_inc(dv, 1)

        @block.scalar
        def _(sce):
            import concourse.mybir as mybir
            for g in range(NG):
                sce.wait_ge(pe, 3 * g + 2)
                sce.activation(out=hT_sb[:, g % 2], in_=ps_h[:],
                               func=mybir.ActivationFunctionType.Relu,
                               bias=b1_sb[:], scale=1.0).then_inc(sc, 1)

    nc.compile()
    return nc


def build_main():
    """NEFF-2 (pipelined): edge remap + mask/ew + x_kept/batch gathers.

    gpsimd: pure indirect-gather issuer (x_kept rows, batch, edge chunks).
    sync:   bounces, reloads, ew loads, output stores.
    vector: per-chunk mask/new_ei/ew arithmetic (double-buffered).
    """
    import concourse.bass as bass
    import concourse.bacc as bacc
    import concourse.mybir as mybir
    from concourse.bass import IndirectOffsetOnAxis
    f32 = mybir.dt.float32
    i32 = mybir.dt.int32
    u8 = mybir.dt.uint8
    Alu = mybir.AluOpType

    ECH2 = 12544              # edges per chunk
    CCOLS = ECH2 // 128       # 98
    NCH2 = ESH // ECH2        # 32 chunks

    nc = bacc.Bacc("TRN2", debug=False)
    x_full = nc.declare_dram_parameter("x_full", [NPAD, D], f32, isOutput=False)
    rtab = nc.declare_dram_parameter("rtab", [NPAD, 1], f32, isOutput=False)
    batch_all = nc.declare_dram_parameter("batch_all", [NPAD, 1], f32, isOutput=False)
    eu_in = nc.declare_dram_parameter("eu_in", [128, EF], i32, isOutput=False)   # wrapped
    ev_in = nc.declare_dram_parameter("ev_in", [128, EF], i32, isOutput=False)   # wrapped
    ew_in = nc.declare_dram_parameter("ew_in", [ESH], f32, isOutput=False)       # flat
    myidx_in = nc.declare_dram_parameter("myidx_in", [128, KS // 128], i32, isOutput=False)

    out_newu = nc.declare_dram_parameter("out_newu", [ESH], i32, isOutput=True)
    out_newv = nc.declare_dram_parameter("out_newv", [ESH], i32, isOutput=True)
    out_mask = nc.declare_dram_parameter("out_mask", [ESH], u8, isOutput=True)
    out_ew = nc.declare_dram_parameter("out_ew", [ESH], f32, isOutput=True)
    out_xk = nc.declare_dram_parameter("out_xk", [KS, D], f32, isOutput=True)
    out_bk = nc.declare_dram_parameter("out_bk", [1, KS], f32, isOutput=True)

    KC = KS // 128   # 49
    stg_du = nc.dram_tensor("stg_du", [ECH2], f32)
    stg_dv = nc.dram_tensor("stg_dv", [ECH2], f32)

    from contextlib import ExitStack
    with ExitStack() as ctx:
        block = ctx.enter_context(nc.Block())
        stage_u = ctx.enter_context(nc.sbuf_tensor("stage_u", [1, ECH2], f32))
        stage_v = ctx.enter_context(nc.sbuf_tensor("stage_v", [1, ECH2], f32))
        eu_sb = ctx.enter_context(nc.sbuf_tensor("eu_sb", [128, EF], i32))
        ev_sb = ctx.enter_context(nc.sbuf_tensor("ev_sb", [128, EF], i32))
        ru = ctx.enter_context(nc.sbuf_tensor("ru", [128, 2, CCOLS], f32))
        rv = ctx.enter_context(nc.sbuf_tensor("rv", [128, 2, CCOLS], f32))
        mu = ctx.enter_context(nc.sbuf_tensor("mu", [128, CCOLS], f32))
        mv = ctx.enter_context(nc.sbuf_tensor("mv", [128, CCOLS], f32))
        t0 = ctx.enter_context(nc.sbuf_tensor("t0", [128, CCOLS], f32))
        nu_i = ctx.enter_context(nc.sbuf_tensor("nu_i", [128, 2, CCOLS], i32))
        nv_i = ctx.enter_context(nc.sbuf_tensor("nv_i", [128, 2, CCOLS], i32))
        mk8 = ctx.enter_context(nc.sbuf_tensor("mk8", [128, 2, CCOLS], u8))
        ewc = ctx.enter_context(nc.sbuf_tensor("ewc", [128, 2, CCOLS], f32))
        ewo = ctx.enter_context(nc.sbuf_tensor("ewo", [128, 2, CCOLS], f32))
        myidx_sb = ctx.enter_context(nc.sbuf_tensor("myidx_sb", [128, KC], i32))
        xg = ctx.enter_context(nc.sbuf_tensor("xg", [128, KC, D], f32))
        bk_sb = ctx.enter_context(nc.sbuf_tensor("bk_sb", [1, KS], f32))
        ld0 = ctx.enter_context(nc.semaphore("ld0"))
        gxk = ctx.enter_context(nc.semaphore("gxk"))
        gu = ctx.enter_context(nc.semaphore("gu"))
        gv = ctx.enter_context(nc.semaphore("gv"))
        bu = ctx.enter_context(nc.semaphore("bu"))
        bv = ctx.enter_context(nc.semaphore("bv"))
        ldm = ctx.enter_context(nc.semaphore("ldm"))
        rdy = ctx.enter_context(nc.semaphore("rdy"))
        vd = ctx.enter_context(nc.semaphore("vd"))
        st = ctx.enter_context(nc.semaphore("st"))
        @block.gpsimd
        def _(g):
            g.wait_ge(ld0, 48)          # eu/ev/myidx loaded by sync
            # x_kept rows + batch gathers
            for j in range(KC):
                g.indirect_dma_start(
                    out=xg[:, j, :], out_offset=None, in_=x_full[:],
                    in_offset=IndirectOffsetOnAxis(ap=myidx_sb[:, j:j + 1], axis=0),
                ).then_inc(gxk, 16)
            g.indirect_dma_start(
                out=bk_sb[:].rearrange("o f -> o f ()"), out_offset=None,
                in_=batch_all[:],
                in_offset=IndirectOffsetOnAxis(ap=myidx_sb[:, :], axis=0),
            ).then_inc(gxk, 16)
            # edge gathers, one instruction per endpoint-chunk
            for k in range(NCH2):
                g.wait_ge(bu, 16 * k)
                g.indirect_dma_start(
                    out=stage_u[:].rearrange("o f -> o f ()"), out_offset=None,
                    in_=rtab[:],
                    in_offset=IndirectOffsetOnAxis(
                        ap=eu_sb[:, k * CCOLS:(k + 1) * CCOLS], axis=0),
                ).then_inc(gu, 16)
                g.wait_ge(bv, 16 * k)
                g.indirect_dma_start(
                    out=stage_v[:].rearrange("o f -> o f ()"), out_offset=None,
                    in_=rtab[:],
                    in_offset=IndirectOffsetOnAxis(
                        ap=ev_sb[:, k * CCOLS:(k + 1) * CCOLS], axis=0),
                ).then_inc(gv, 16)

        @block.sync
        def _(s):
            s.dma_start(out=eu_sb[:], in_=eu_in[:]).then_inc(ld0, 16)
            s.dma_start(out=ev_sb[:], in_=ev_in[:]).then_inc(ld0, 16)
            s.dma_start(out=myidx_sb[:], in_=myidx_in[:]).then_inc(ld0, 16)
            for k in range(NCH2):
                b = k % 2
                s.wait_ge(gu, 16 * (k + 1))
                s.dma_start(out=stg_du[None, :], in_=stage_u[:]).then_inc(bu, 16)
                s.wait_ge(bu, 16 * (k + 1))
                s.dma_start(out=ru[:, b],
                            in_=stg_du[:].rearrange("(p f) -> p f", p=128)).then_inc(ldm, 16)
                s.wait_ge(gv, 16 * (k + 1))
                s.dma_start(out=stg_dv[None, :], in_=stage_v[:]).then_inc(bv, 16)
                s.wait_ge(bv, 16 * (k + 1))
                s.dma_start(out=rv[:, b],
                            in_=stg_dv[:].rearrange("(p f) -> p f", p=128)).then_inc(ldm, 16)
                s.dma_start(out=ewc[:, b],
                            in_=ew_in[k * ECH2:(k + 1) * ECH2].rearrange(
                                "(p f) -> p f", p=128)).then_inc(ldm, 16)
                s.wait_ge(ldm, 48 * (k + 1))
                s.sem_inc(rdy, 1)
                if k >= 1:
                    bb = (k - 1) % 2
                    s.wait_ge(vd, k)
                    sl = slice((k - 1) * ECH2, k * ECH2)
                    s.dma_start(out=out_newu[sl].rearrange("(p f) -> p f", p=128),
                                in_=nu_i[:, bb]).then_inc(st, 16)
                    s.dma_start(out=out_newv[sl].rearrange("(p f) -> p f", p=128),
                                in_=nv_i[:, bb]).then_inc(st, 16)
                    s.dma_start(out=out_mask[sl].rearrange("(p f) -> p f", p=128),
                                in_=mk8[:, bb]).then_inc(st, 16)
                    s.dma_start(out=out_ew[sl].rearrange("(p f) -> p f", p=128),
                                in_=ewo[:, bb]).then_inc(st, 16)
            # final chunk stores + x_kept/batch writes
            bb = (NCH2 - 1) % 2
            s.wait_ge(vd, NCH2)
            sl = slice((NCH2 - 1) * ECH2, NCH2 * ECH2)
            s.dma_start(out=out_newu[sl].rearrange("(p f) -> p f", p=128),
                        in_=nu_i[:, bb]).then_inc(st, 16)
            s.dma_start(out=out_newv[sl].rearrange("(p f) -> p f", p=128),
                        in_=nv_i[:, bb]).then_inc(st, 16)
            s.dma_start(out=out_mask[sl].rearrange("(p f) -> p f", p=128),
                        in_=mk8[:, bb]).then_inc(st, 16)
            s.dma_start(out=out_ew[sl].rearrange("(p f) -> p f", p=128),
                        in_=ewo[:, bb]).then_inc(st, 16)
            s.wait_ge(gxk, 16 * (KC + 1))
            s.dma_start(out=out_bk[:], in_=bk_sb[:]).then_inc(st, 16)
            s.dma_start(out=out_xk[:].rearrange("(f p) d -> p f d", p=128),
                        in_=xg[:]).then_inc(st, 16)
            s.wait_ge(st, 16 * (4 * NCH2 + 2))

        @block.vector
        def _(v):
            for k in range(NCH2):
                b = k % 2
                v.wait_ge(rdy, k + 1)
                if k >= 2:
                    v.wait_ge(st, 64 * (k - 1))   # out tiles [b] stored
                v.tensor_scalar(out=mu[:], in0=ru[:, b], scalar1=float(K),
                                scalar2=None, op0=Alu.is_lt)
                v.tensor_scalar(out=mv[:], in0=rv[:, b], scalar1=float(K),
                                scalar2=None, op0=Alu.is_lt)
                v.tensor_scalar(out=t0[:], in0=ru[:, b], scalar1=1.0,
                                scalar2=None, op0=Alu.add)
                v.tensor_tensor(out=t0[:], in0=t0[:], in1=mu[:], op=Alu.mult)
                v.tensor_scalar(out=t0[:], in0=t0[:], scalar1=1.0,
                                scalar2=None, op0=Alu.subtract)
                v.tensor_copy(out=nu_i[:, b], in_=t0[:])
                v.tensor_scalar(out=t0[:], in0=rv[:, b], scalar1=1.0,
                                scalar2=None, op0=Alu.add)
                v.tensor_tensor(out=t0[:], in0=t0[:], in1=mv[:], op=Alu.mult)
                v.tensor_scalar(out=t0[:], in0=t0[:], scalar1=1.0,
                                scalar2=None, op0=Alu.subtract)
                v.tensor_copy(out=nv_i[:, b], in_=t0[:])
                v.tensor_tensor(out=t0[:], in0=mu[:], in1=mv[:], op=Alu.mult)
                v.tensor_copy(out=mk8[:, b], in_=t0[:])
                v.tensor_tensor(out=ewo[:, b], in0=ewc[:, b], in1=t0[:], op=Alu.mult)
                v.sem_inc(vd, 1)

    nc.compile()
    return nc


def kernel(x, edge_index, edge_weight, batch, W1, b1, W2, b2):
    _install_profile_hook()
    from concourse.bass_utils import run_bass_kernel_spmd

    x = np.ascontiguousarray(np.asarray(x, np.float32))
    edge_index = np.asarray(edge_index)
    edge_weight = np.asarray(edge_weight, np.float32)
    batch = np.asarray(batch)
    W1 = np.asarray(W1, np.float32)
    b1 = np.asarray(b1, np.float32)
    W2 = np.asarray(W2, np.float32)
    b2 = np.asarray(b2, np.float32)

    # ---------------- NEFF-1: scores ----------------
    if "scores" not in _BUILT:
        _BUILT["scores"] = build_scores()
    nc1 = _BUILT["scores"]

    x_pad = np.zeros((NPAD, D), np.float32)
    x_pad[:N] = x
    w1t_np = np.ascontiguousarray(W1.T)                 # [256, 64]
    b1_np = np.ascontiguousarray(b1.reshape(H, 1))
    w2t_np = np.ascontiguousarray(W2.reshape(1, H).T)   # [64, 1]
    b2_np = np.ascontiguousarray(b2.reshape(1, 1))
    ident = np.eye(128, dtype=np.float32)

    in1 = []
    for c in range(NCORES):
        in1.append({
            "x_sh": x_pad[c * SH:(c + 1) * SH],
            "w1t": w1t_np, "b1_in": b1_np, "w2t": w2t_np, "b2_in": b2_np,
            "ident": ident,
        })
    import os
    _tr = os.environ.get("KTRACE", "0") == "1"
    res1 = run_bass_kernel_spmd(nc1, in1, core_ids=list(range(NCORES)), trace=_tr)
    # scores: out_s [128, NT] wrapped: node c*SH + t*128 + p at [p, t]
    scores = np.empty(NPAD, np.float32)
    for c in range(NCORES):
        sw = np.asarray(res1.results[c]["out_s"])       # [128, NT]
        scores[c * SH:(c + 1) * SH] = sw.T.reshape(-1)  # i = t*128 + p
    scores[:N] = (scores[:N] + b2.reshape(1)[0]).astype(np.float32)
    ktime1 = getattr(res1, "exec_time_ns", None)

    # ---------------- host: rank tables ----------------
    bits = scores.view(np.int32).astype(np.int64)
    key = (bits ^ ((bits >> 31) | np.int64(-0x80000000))) & 0xFFFFFFFF
    gid = np.arange(NPAD, dtype=np.int64)
    key = np.where(gid < N, key, gid - N)     # pads: distinct tiny keys
    order = np.lexsort((gid, -key))           # descending key, ties by id
    rank = np.empty(NPAD, np.int64)
    rank[order] = gid
    inv = order                               # rank -> node id

    # ---------------- NEFF-2: gathers ----------------
    if "main" not in _BUILT:
        _BUILT["main"] = build_main()
    nc2 = _BUILT["main"]

    ei = edge_index.astype(np.int64)
    eu_flat = np.zeros(EPAD, np.int32)
    ev_flat = np.zeros(EPAD, np.int32)
    ew_flat = np.zeros(EPAD, np.float32)
    eu_flat[:E] = ei[0]
    ev_flat[:E] = ei[1]
    ew_flat[:E] = edge_weight
    batch_pad = np.zeros((NPAD, 1), np.float32)
    batch_pad[:N, 0] = batch.astype(np.float32)
    rtab_np = rank.astype(np.float32).reshape(NPAD, 1)

    in2 = []
    for c in range(NCORES):
        sl = slice(c * ESH, (c + 1) * ESH)
        eu_sh = eu_flat[sl].reshape(ESH // 128, 128).T.copy()   # wrapped
        ev_sh = ev_flat[sl].reshape(ESH // 128, 128).T.copy()
        # ew must be chunk-row-major: edge (k*ECH + p*196 + f) at [p, k*196+f]
        myidx = inv[c * 6250: c * 6250 + KS].astype(np.int32)
        myidx_w = myidx.reshape(KS // 128, 128).T.copy()
        in2.append({
            "x_full": x_pad, "rtab": rtab_np, "batch_all": batch_pad,
            "eu_in": eu_sh, "ev_in": ev_sh, "ew_in": ew_flat[sl].copy(),
            "myidx_in": myidx_w,
        })
    res2 = run_bass_kernel_spmd(nc2, in2, core_ids=list(range(NCORES)), trace=_tr)
    ktime2 = getattr(res2, "exec_time_ns", None)
    kernel.exec_times = (ktime1, ktime2)

    # ---------------- assemble ----------------
    new_ei = np.empty((2, E), np.int32)
    mask = np.empty(E, bool)
    ew_out = np.empty(E, np.float32)
    x_kept = np.empty((K, D), np.float32)
    batch_kept = np.empty(K, batch.dtype)
    for c in range(NCORES):
        r = res2.results[c]
        lo, hi = c * ESH, min((c + 1) * ESH, E)
        n = hi - lo
        if n > 0:
            new_ei[0, lo:hi] = np.asarray(r["out_newu"])[:n]
            new_ei[1, lo:hi] = np.asarray(r["out_newv"])[:n]
            mask[lo:hi] = np.asarray(r["out_mask"])[:n] != 0
            ew_out[lo:hi] = np.asarray(r["out_ew"])[:n]
        # kept rows: out_xk row j = rank c*6250 + j
        klo = c * 6250
        kn = min(6250, K - klo)
        x_kept[klo:klo + kn] = np.asarray(r["out_xk"])[:kn]
        batch_kept[klo:klo + kn] = np.asarray(r["out_bk"])[0, :kn].astype(batch.dtype)
    indices = inv[:K].astype(np.int32)
    return x_kept, new_ei, ew_out, mask, batch_kept, indices
